# revision 40
# baseline (speedup 1.0000x reference)
"""Trainium2 Bass kernel for gnn_message_passing (gather + matmul).

Reference computation:
    out[b, m, p] = sum_{c,k} W[m, c*KS+k] * x[b, c, idx[p, k]]
with B=32, C=32, P=4096 pixels, KS=9 neighbors, K=64 output channels.

Strategy (8 NeuronCores, pixel-parallel with a replicated token table):
  The gather is the expensive part: SWDGE descriptor generation on the
  GPSIMD Q7 costs ~10ns per gathered token, so we minimize token COUNT by
  maximizing token SIZE.  idx is shared by every (batch, channel), so a
  token for pixel q packs all 32*32 = 1024 values x[:, :, q] (2KB bf16).

  Per core:
   - DMA the FULL x (1024, 4096) f32 in 8 slabs of 128 (b,c) rows with an
     f32->bf16 cast riding the SWDGE DMA; PE-transpose 128x128 blocks into
     PSUM and copy into an SBUF-resident token table
     T[q%128, q//128, bc] (2KB contiguous per token -> one descriptor).
   - dma_gather (SBUF source, transpose=True, elem=1024, tokens_per_rank=
     128, 2KB payload/rank) with int16 index lists for THIS core's 512
     pixels: 9 calls x 512 idxs spread over 4 SWDGE queues.  Gathered
     G[p128, k, f, i] = x[bc=f*128+p128, idx[pix_i, k]] - the matmul rhs
     with the contraction (b%4, c) on partitions, batch-group f on free.
   - Matmuls with block-diagonal weights: lhsT BD[bp,k] (128x128 bf16)
     maps rhs partitions (b', c) -> out partitions (j, m) for batches
     4f+2bp+j, accumulating the 9 k's in PSUM (f32).  Same BD reused for
     every batch group f.
   - PSUM -> SBUF (DVE) -> DRAM out (2048, 512) f32 = (f, bp, j, m) x pix.

  Numbers that shaped this design (measured on HW via neuron-profile):
   - dma_gather Q7 desc-gen ~10ns/token and ~1K descriptors max per call
     (bigger calls crash the runtime); token count is the knob that
     matters, hence full-x replication for 2KB tokens (4608/core).
   - SBUF-source single-stripe gather avoids an 8MB DRAM table write +
     9.4MB HBM random reads.
"""

import os

import numpy as np
import ml_dtypes

import concourse.bass as bass
import concourse.mybir as mybir
import concourse.tile as tile
from concourse import bacc
from concourse.bass_utils import run_bass_kernel_spmd

B, C, H, W_IMG = 32, 32, 64, 64
P = H * W_IMG          # 4096 pixels
KS = 9                 # neighbors per pixel
K = 64                 # output channels
NCORES = 8
PPC = P // NCORES      # 512 pixels per core
NBC = B * C            # 1024 = full (b, c) dim
NSLAB = NBC // 128     # 8 slabs
NF = NSLAB             # 8 batch groups of 4 on the gather free dim
# 4 SWDGE queues overlap gather desc-gen on HW; CoreSim's queue-sem model
# rejects it, so sim validation sets KERNEL_NQUEUES=1.
NQUEUES = int(os.environ.get("KERNEL_NQUEUES", "4"))

_cache = {}


def _build():
    nc = bacc.Bacc("TRN2", target_bir_lowering=False, debug=False,
                   num_devices=NCORES, num_swdge_queues=NQUEUES)

    x_ext = nc.dram_tensor("x", [NBC, P], mybir.dt.float32,
                           kind="ExternalInput")
    wbd_ext = nc.dram_tensor("wbd", [128, 2 * KS * 128], mybir.dt.bfloat16,
                             kind="ExternalInput")
    idx_ext = nc.dram_tensor("idx16", [128, KS * PPC // 16], mybir.dt.int16,
                             kind="ExternalInput")
    out_ext = nc.dram_tensor("out", [B * K, PPC],
                             mybir.dt.float32, kind="ExternalOutput")
    # DRAM twin of the token table: 5 of the 9 gathers read HBM so the
    # gather transfers use both the HBM and SBUF-fabric bandwidth pools.
    tbl = nc.dram_tensor("tbl", [P, NBC], mybir.dt.bfloat16)

    with tile.TileContext(nc) as tc:
        with (
            tc.tile_pool(name="persist", bufs=1) as pp,
            tc.tile_pool(name="slab", bufs=3) as slp,
            tc.tile_pool(name="stage", bufs=3) as sp,
        ):
            idx_t = pp.tile([128, KS * PPC // 16], mybir.dt.int16, tag="idx")
            bd_t = pp.tile([128, 2 * KS, 128], mybir.dt.bfloat16, tag="bd")
            ident = pp.tile([128, 128], mybir.dt.bfloat16, tag="ident")
            G = pp.tile([128, KS, NF, PPC], mybir.dt.bfloat16, tag="G")
            # SBUF-resident token table: token q = T[q%128, q//128, :]
            # (2KB contiguous -> single-stripe single-descriptor gather)
            T = pp.tile([128, P // 128, NBC], mybir.dt.bfloat16, tag="T")

            nc.sync.dma_start(idx_t[:], idx_ext[:, :])
            nc.sync.dma_start(bd_t[:], wbd_ext[:, :].rearrange(
                "p (a b) -> p a b", b=128))

            from concourse.masks import make_identity
            make_identity(nc, ident[:])

            # token table: T[q%128, q//128, bc] = x[bc, q] (bf16).  The
            # f32->bf16 cast rides the input DMA (SWDGE cast); transposes
            # run in bf16 on the PE; the table stays in SBUF.
            with tc.tile_pool(name="pstr", bufs=4, space="PSUM") as ptr:
                for s in range(NSLAB):
                    Xs = slp.tile([128, P], mybir.dt.bfloat16, tag="Xs")
                    nc.gpsimd.dma_start(Xs[:],
                                        x_ext[s * 128:(s + 1) * 128, :])
                    for g in range(8):
                        pt = ptr.tile([128, 4, 128], mybir.dt.bfloat16,
                                      tag="pt")
                        for r4 in range(4):
                            r = g * 4 + r4
                            nc.tensor.transpose(
                                pt[:, r4, :],
                                Xs[:, r * 128:(r + 1) * 128],
                                ident[:])
                        nc.vector.tensor_copy(
                            out=T[:, g * 4:(g + 1) * 4,
                                  s * 128:(s + 1) * 128],
                            in_=pt[:])


            # Mirror the finished table to DRAM in one big 2KB-desc DMA;
            # the last two gathers read it from HBM so their transfers use
            # the HBM pool while the SBUF-fabric gathers still drain.
            nc.sync.dma_start(
                tbl[:, :].rearrange("(r p) e -> p r e", p=128), T[:])

            # gather: G[p, k, f, i] = token(idx[pix_i, k])[f*128+p].
            for k in range(KS):
                if k >= 7:
                    nc.gpsimd.dma_gather(
                        G[:, k, :, :],
                        tbl[:, :],
                        idx_t[:, k * (PPC // 16):(k + 1) * (PPC // 16)],
                        PPC, PPC, NBC,
                        transpose=True,
                        queue_num=k % NQUEUES,
                    )
                else:
                    nc.gpsimd.dma_gather(
                        G[:, k, :, :],
                        T[:].rearrange("p r e -> p (r e)"),
                        idx_t[:, k * (PPC // 16):(k + 1) * (PPC // 16)],
                        PPC, PPC, NBC,
                        transpose=True,
                        sbuf_tokens_per_rank=128,
                        sbuf_free_dim_per_rank=2 * NBC,
                        queue_num=k % NQUEUES,
                    )

            # Keep the PE's HAM clock warm through the gather window: the
            # PE idles ~65..85us otherwise and drops to 1.2GHz for the
            # matmul phase.  Dummy transposes read the last table block so
            # they become runnable exactly when the build finishes.
            with tc.tile_pool(name="pswarm", bufs=1, space="PSUM") as pw:
                junk = pw.tile([128, 128], mybir.dt.bfloat16, tag="junk")
                for _ in range(40):
                    nc.tensor.transpose(junk[:], T[:, 31, 896:1024],
                                        ident[:])

            # matmuls: batch group f, pair bp -> batches 4f+2bp+{0,1}.
            # f-major so each (f, bp) group's PSUM copy + out DMA stagger
            # into the matmul stream instead of bunching at the end.
            with tc.tile_pool(name="psmm", bufs=6, space="PSUM") as pmm:
                for f in range(NF):
                    for bp in range(2):
                        ps = pmm.tile([128, PPC], mybir.dt.float32,
                                      tag="ps_mm")
                        for k in range(KS):
                            nc.tensor.matmul(
                                ps[:],
                                bd_t[:, bp * KS + k, :],
                                G[:, k, f, :],
                                start=(k == 0),
                                stop=(k == KS - 1),
                            )
                        st = sp.tile([128, PPC], mybir.dt.float32,
                                     tag="st")
                        nc.vector.tensor_copy(out=st[:], in_=ps[:])
                        row = (f * 2 + bp) * 128
                        nc.sync.dma_start(out_ext[row:row + 128, :], st[:])

    nc.compile()
    return nc


def _get_nc():
    if "nc" not in _cache:
        _cache["nc"] = _build()
    return _cache["nc"]


def _prep_idx16(idx: np.ndarray) -> list:
    """idx (1,64,64,9) int32 -> per-core (128, KS*PPC//16) int16 lists.

    Core i handles pixels [PPC*i, PPC*(i+1)).  Chunk k holds idx[p, k] for
    those pixels, wrapped: element j at partition j%16, col j//16
    (replicated to the 8 16-partition groups)."""
    lst = idx.reshape(P, KS).astype(np.int16)
    outs = []
    for i in range(NCORES):
        o = np.zeros((128, KS * (PPC // 16)), dtype=np.int16)
        for k in range(KS):
            w = lst[PPC * i:PPC * (i + 1), k].reshape(PPC // 16, 16).T
            o[:, k * (PPC // 16):(k + 1) * (PPC // 16)] = np.tile(w, (8, 1))
        outs.append(o)
    return outs


def _prep_wbd(weights: np.ndarray) -> np.ndarray:
    """weights (64, 288) f32 -> block-diag lhsT set (128, 2*KS*128) bf16.

    BD[bp, k][32*b' + c, 64*j + m] = W[m, c*KS+k] if b' == 2*bp+j else 0,
    for b' in 0..4 (batch-within-group); reused for every group f."""
    bd = np.zeros((2, KS, 128, 128), dtype=np.float32)
    for k in range(KS):
        wk = weights[:, k::KS]  # (64, 32) = W[m, c*KS+k]
        for bp in range(2):
            for j in range(2):
                bprime = 2 * bp + j
                bd[bp, k, 32 * bprime:32 * bprime + 32, 64 * j:64 * j + 64] = \
                    wk.T
    return bd.reshape(2 * KS, 128, 128).transpose(1, 0, 2).reshape(
        128, 2 * KS * 128).astype(ml_dtypes.bfloat16)


def prep_in_maps(x: np.ndarray, weights: np.ndarray, idx: np.ndarray):
    idx16s = _prep_idx16(np.asarray(idx))
    wbd = _prep_wbd(np.asarray(weights, dtype=np.float32))
    xf = np.ascontiguousarray(
        np.asarray(x, dtype=np.float32).reshape(NBC, P))
    return [{"x": xf, "wbd": wbd, "idx16": idx16s[i]} for i in range(NCORES)]


def assemble_out(results) -> np.ndarray:
    out = np.empty((B, K, P), dtype=np.float32)
    for i in range(NCORES):
        r = np.asarray(results[i]["out"]).astype(np.float32).reshape(
            NF, 2, 2, K, PPC)  # (f, bp, j, m, p)
        for f in range(NF):
            for bp in range(2):
                for j in range(2):
                    out[4 * f + 2 * bp + j, :, PPC * i:PPC * (i + 1)] = \
                        r[f, bp, j]
    return out.reshape(B, K, H, W_IMG)


last_results = None


def kernel(x, weights, idx):
    global last_results
    nc = _get_nc()
    in_maps = prep_in_maps(x, weights, idx)
    trace = bool(int(os.environ.get("KERNEL_TRACE", "0")))
    res = run_bass_kernel_spmd(nc, in_maps, core_ids=list(range(NCORES)),
                               trace=trace)
    last_results = res
    return assemble_out(res.results)


# revision 42
# speedup vs baseline: 1.0943x; 1.0943x over previous
"""Trainium2 Bass kernel for gnn_message_passing (gather + matmul).

Reference computation:
    out[b, m, p] = sum_{c,k} W[m, c*KS+k] * x[b, c, idx[p, k]]
with B=32, C=32, P=4096 pixels, KS=9 neighbors, K=64 output channels.

Strategy (8 NeuronCores, pixel-parallel with a replicated token table):
  The gather is the expensive part: SWDGE descriptor generation on the
  GPSIMD Q7 costs ~10ns per gathered token, so we minimize token COUNT by
  maximizing token SIZE.  idx is shared by every (batch, channel), so a
  token for pixel q packs all 32*32 = 1024 values x[:, :, q] (2KB bf16).

  Per core:
   - DMA the FULL x (1024, 4096) f32 in 8 slabs of 128 (b,c) rows with an
     f32->bf16 cast riding the SWDGE DMA; PE-transpose 128x128 blocks into
     PSUM and copy into an SBUF-resident token table
     T[q%128, q//128, bc] (2KB contiguous per token -> one descriptor).
   - dma_gather (SBUF source, transpose=True, elem=1024, tokens_per_rank=
     128, 2KB payload/rank) with int16 index lists for THIS core's 512
     pixels: 9 calls x 512 idxs spread over 4 SWDGE queues.  Gathered
     G[p128, k, f, i] = x[bc=f*128+p128, idx[pix_i, k]] - the matmul rhs
     with the contraction (b%4, c) on partitions, batch-group f on free.
   - Matmuls with block-diagonal weights: lhsT BD[bp,k] (128x128 bf16)
     maps rhs partitions (b', c) -> out partitions (j, m) for batches
     4f+2bp+j, accumulating the 9 k's in PSUM (f32).  Same BD reused for
     every batch group f.
   - PSUM -> SBUF (DVE) -> DRAM out (2048, 512) f32 = (f, bp, j, m) x pix.

  Numbers that shaped this design (measured on HW via neuron-profile):
   - dma_gather Q7 desc-gen ~10ns/token and ~1K descriptors max per call
     (bigger calls crash the runtime); token count is the knob that
     matters, hence full-x replication for 2KB tokens (4608/core).
   - SBUF-source single-stripe gather avoids an 8MB DRAM table write +
     9.4MB HBM random reads.
"""

import os

import numpy as np
import ml_dtypes

import concourse.bass as bass
import concourse.mybir as mybir
import concourse.tile as tile
from concourse import bacc
from concourse.bass_utils import run_bass_kernel_spmd

B, C, H, W_IMG = 32, 32, 64, 64
P = H * W_IMG          # 4096 pixels
KS = 9                 # neighbors per pixel
K = 64                 # output channels
NCORES = 8
PPC = P // NCORES      # 512 pixels per core
NBC = B * C            # 1024 = full (b, c) dim
NSLAB = NBC // 128     # 8 slabs
NF = NSLAB             # 8 batch groups of 4 on the gather free dim
# 4 SWDGE queues overlap gather desc-gen on HW; CoreSim's queue-sem model
# rejects it, so sim validation sets KERNEL_NQUEUES=1.
NQUEUES = int(os.environ.get("KERNEL_NQUEUES", "4"))

_cache = {}


def _build():
    nc = bacc.Bacc("TRN2", target_bir_lowering=False, debug=False,
                   num_devices=NCORES, num_swdge_queues=NQUEUES)

    x_ext = nc.dram_tensor("x", [NBC, P], mybir.dt.float32,
                           kind="ExternalInput")
    wbd_ext = nc.dram_tensor("wbd", [128, 2 * KS * 128], mybir.dt.bfloat16,
                             kind="ExternalInput")
    idx_ext = nc.dram_tensor("idx16", [128, KS * PPC // 16], mybir.dt.int16,
                             kind="ExternalInput")
    out_ext = nc.dram_tensor("out", [B * K, PPC],
                             mybir.dt.float32, kind="ExternalOutput")

    with tile.TileContext(nc) as tc:
        with (
            tc.tile_pool(name="persist", bufs=1) as pp,
            tc.tile_pool(name="slab", bufs=3) as slp,
            tc.tile_pool(name="stage", bufs=3) as sp,
        ):
            idx_t = pp.tile([128, KS * PPC // 16], mybir.dt.int16, tag="idx")
            bd_t = pp.tile([128, 2 * KS, 128], mybir.dt.bfloat16, tag="bd")
            ident = pp.tile([128, 128], mybir.dt.bfloat16, tag="ident")
            G = pp.tile([128, KS, NF, PPC], mybir.dt.bfloat16, tag="G")
            # SBUF-resident token table: token q = T[q%128, q//128, :]
            # (2KB contiguous -> single-stripe single-descriptor gather)
            T = pp.tile([128, P // 128, NBC], mybir.dt.bfloat16, tag="T")

            nc.sync.dma_start(idx_t[:], idx_ext[:, :])
            nc.sync.dma_start(bd_t[:], wbd_ext[:, :].rearrange(
                "p (a b) -> p a b", b=128))

            from concourse.masks import make_identity
            make_identity(nc, ident[:])

            # token table: T[q%128, q//128, bc] = x[bc, q] (bf16).  The
            # f32->bf16 cast rides the input DMA (SWDGE cast); transposes
            # run in bf16 on the PE; the table stays in SBUF.
            with tc.tile_pool(name="pstr", bufs=4, space="PSUM") as ptr:
                for s in range(NSLAB):
                    Xs = slp.tile([128, P], mybir.dt.bfloat16, tag="Xs")
                    nc.gpsimd.dma_start(Xs[:],
                                        x_ext[s * 128:(s + 1) * 128, :])
                    for g in range(8):
                        pt = ptr.tile([128, 4, 128], mybir.dt.bfloat16,
                                      tag="pt")
                        for r4 in range(4):
                            r = g * 4 + r4
                            nc.tensor.transpose(
                                pt[:, r4, :],
                                Xs[:, r * 128:(r + 1) * 128],
                                ident[:])
                        nc.vector.tensor_copy(
                            out=T[:, g * 4:(g + 1) * 4,
                                  s * 128:(s + 1) * 128],
                            in_=pt[:])


            # gather: G[p, k, f, i] = token(idx[pix_i, k])[f*128+p]
            for k in range(KS):
                nc.gpsimd.dma_gather(
                    G[:, k, :, :],
                    T[:].rearrange("p r e -> p (r e)"),
                    idx_t[:, k * (PPC // 16):(k + 1) * (PPC // 16)],
                    PPC,        # num_idxs
                    PPC,        # num_idxs_reg (all valid)
                    NBC,        # elem_size (bf16 elements = 2KB)
                    transpose=True,
                    sbuf_tokens_per_rank=128,
                    sbuf_free_dim_per_rank=2 * NBC,  # payload bytes per rank
                    queue_num=k % NQUEUES,
                )

            # Keep the PE's HAM clock warm through the gather window: the
            # PE idles ~65..85us otherwise and drops to 1.2GHz for the
            # matmul phase.  Dummy transposes read the last table block so
            # they become runnable exactly when the build finishes.
            with tc.tile_pool(name="pswarm", bufs=1, space="PSUM") as pw:
                junk = pw.tile([128, 128], mybir.dt.bfloat16, tag="junk")
                for _ in range(40):
                    nc.tensor.transpose(junk[:], T[:, 31, 896:1024],
                                        ident[:])

            # matmuls: batch group f, pair bp -> batches 4f+2bp+{0,1}.
            # f-major so each (f, bp) group's PSUM copy + out DMA stagger
            # into the matmul stream instead of bunching at the end.
            with tc.tile_pool(name="psmm", bufs=6, space="PSUM") as pmm:
                for f in range(NF):
                    for bp in range(2):
                        ps = pmm.tile([128, PPC], mybir.dt.float32,
                                      tag="ps_mm")
                        for k in range(KS):
                            nc.tensor.matmul(
                                ps[:],
                                bd_t[:, bp * KS + k, :],
                                G[:, k, f, :],
                                start=(k == 0),
                                stop=(k == KS - 1),
                            )
                        st = sp.tile([128, PPC], mybir.dt.float32,
                                     tag="st")
                        nc.vector.tensor_copy(out=st[:], in_=ps[:])
                        row = (f * 2 + bp) * 128
                        nc.sync.dma_start(out_ext[row:row + 128, :], st[:])

    nc.compile()
    return nc


def _get_nc():
    if "nc" not in _cache:
        _cache["nc"] = _build()
    return _cache["nc"]


def _prep_idx16(idx: np.ndarray) -> list:
    """idx (1,64,64,9) int32 -> per-core (128, KS*PPC//16) int16 lists.

    Core i handles pixels [PPC*i, PPC*(i+1)).  Chunk k holds idx[p, k] for
    those pixels, wrapped: element j at partition j%16, col j//16
    (replicated to the 8 16-partition groups)."""
    lst = idx.reshape(P, KS).astype(np.int16)
    outs = []
    for i in range(NCORES):
        o = np.zeros((128, KS * (PPC // 16)), dtype=np.int16)
        for k in range(KS):
            w = lst[PPC * i:PPC * (i + 1), k].reshape(PPC // 16, 16).T
            o[:, k * (PPC // 16):(k + 1) * (PPC // 16)] = np.tile(w, (8, 1))
        outs.append(o)
    return outs


def _prep_wbd(weights: np.ndarray) -> np.ndarray:
    """weights (64, 288) f32 -> block-diag lhsT set (128, 2*KS*128) bf16.

    BD[bp, k][32*b' + c, 64*j + m] = W[m, c*KS+k] if b' == 2*bp+j else 0,
    for b' in 0..4 (batch-within-group); reused for every group f."""
    bd = np.zeros((2, KS, 128, 128), dtype=np.float32)
    for k in range(KS):
        wk = weights[:, k::KS]  # (64, 32) = W[m, c*KS+k]
        for bp in range(2):
            for j in range(2):
                bprime = 2 * bp + j
                bd[bp, k, 32 * bprime:32 * bprime + 32, 64 * j:64 * j + 64] = \
                    wk.T
    return bd.reshape(2 * KS, 128, 128).transpose(1, 0, 2).reshape(
        128, 2 * KS * 128).astype(ml_dtypes.bfloat16)


def prep_in_maps(x: np.ndarray, weights: np.ndarray, idx: np.ndarray):
    idx16s = _prep_idx16(np.asarray(idx))
    wbd = _prep_wbd(np.asarray(weights, dtype=np.float32))
    xf = np.ascontiguousarray(
        np.asarray(x, dtype=np.float32).reshape(NBC, P))
    return [{"x": xf, "wbd": wbd, "idx16": idx16s[i]} for i in range(NCORES)]


def assemble_out(results) -> np.ndarray:
    out = np.empty((B, K, P), dtype=np.float32)
    for i in range(NCORES):
        r = np.asarray(results[i]["out"]).astype(np.float32).reshape(
            NF, 2, 2, K, PPC)  # (f, bp, j, m, p)
        for f in range(NF):
            for bp in range(2):
                for j in range(2):
                    out[4 * f + 2 * bp + j, :, PPC * i:PPC * (i + 1)] = \
                        r[f, bp, j]
    return out.reshape(B, K, H, W_IMG)


last_results = None


def kernel(x, weights, idx):
    global last_results
    nc = _get_nc()
    in_maps = prep_in_maps(x, weights, idx)
    trace = bool(int(os.environ.get("KERNEL_TRACE", "0")))
    res = run_bass_kernel_spmd(nc, in_maps, core_ids=list(range(NCORES)),
                               trace=trace)
    last_results = res
    return assemble_out(res.results)


# revision 45
# speedup vs baseline: 1.1960x; 1.0929x over previous
"""Trainium2 Bass kernel for gnn_message_passing (gather + matmul).

Reference computation:
    out[b, m, p] = sum_{c,k} W[m, c*KS+k] * x[b, c, idx[p, k]]
with B=32, C=32, P=4096 pixels, KS=9 neighbors, K=64 output channels.

Strategy (8 NeuronCores, pixel-parallel with a replicated token table):
  The gather is the expensive part: SWDGE descriptor generation on the
  GPSIMD Q7 costs ~10ns per gathered token, so we minimize token COUNT by
  maximizing token SIZE.  idx is shared by every (batch, channel), so a
  token for pixel q packs all 32*32 = 1024 values x[:, :, q] (2KB bf16).

  Per core:
   - DMA the FULL x (1024, 4096) f32 in 8 slabs of 128 (b,c) rows with an
     f32->bf16 cast riding the SWDGE DMA; PE-transpose 128x128 blocks into
     PSUM and copy into an SBUF-resident token table
     T[q%128, q//128, bc] (2KB contiguous per token -> one descriptor).
   - dma_gather (SBUF source, transpose=True, elem=1024, tokens_per_rank=
     128, 2KB payload/rank) with int16 index lists for THIS core's 512
     pixels: 9 calls x 512 idxs spread over 4 SWDGE queues.  Gathered
     G[p128, k, f, i] = x[bc=f*128+p128, idx[pix_i, k]] - the matmul rhs
     with the contraction (b%4, c) on partitions, batch-group f on free.
   - Matmuls with block-diagonal weights: lhsT BD[bp,k] (128x128 bf16)
     maps rhs partitions (b', c) -> out partitions (j, m) for batches
     4f+2bp+j, accumulating the 9 k's in PSUM (f32).  Same BD reused for
     every batch group f.
   - PSUM -> SBUF (DVE) -> DRAM out (2048, 512) f32 = (f, bp, j, m) x pix.

  Numbers that shaped this design (measured on HW via neuron-profile):
   - dma_gather Q7 desc-gen ~10ns/token and ~1K descriptors max per call
     (bigger calls crash the runtime); token count is the knob that
     matters, hence full-x replication for 2KB tokens (4608/core).
   - SBUF-source single-stripe gather avoids an 8MB DRAM table write +
     9.4MB HBM random reads.
"""

import os

import numpy as np
import ml_dtypes

import concourse.bass as bass
import concourse.mybir as mybir
import concourse.tile as tile
from concourse import bacc
from concourse.bass_utils import run_bass_kernel_spmd

B, C, H, W_IMG = 32, 32, 64, 64
P = H * W_IMG          # 4096 pixels
KS = 9                 # neighbors per pixel
K = 64                 # output channels
NCORES = 8
PPC = P // NCORES      # 512 pixels per core
NBC = B * C            # 1024 = full (b, c) dim
NSLAB = NBC // 128     # 8 slabs
NF = NSLAB             # 8 batch groups of 4 on the gather free dim
# 4 SWDGE queues overlap gather desc-gen on HW; CoreSim's queue-sem model
# rejects it, so sim validation sets KERNEL_NQUEUES=1.
NQUEUES = int(os.environ.get("KERNEL_NQUEUES", "4"))

_cache = {}


def _build():
    nc = bacc.Bacc("TRN2", target_bir_lowering=False, debug=False,
                   num_devices=NCORES, num_swdge_queues=NQUEUES)

    x_ext = nc.dram_tensor("x", [NBC, P], mybir.dt.float32,
                           kind="ExternalInput")
    wbd_ext = nc.dram_tensor("wbd", [128, 2 * KS * 128], mybir.dt.bfloat16,
                             kind="ExternalInput")
    idx_ext = nc.dram_tensor("idx16", [128, KS * PPC // 16], mybir.dt.int16,
                             kind="ExternalInput")
    out_ext = nc.dram_tensor("out", [B * K, PPC],
                             mybir.dt.float32, kind="ExternalOutput")

    with tile.TileContext(nc) as tc:
        with (
            tc.tile_pool(name="persist", bufs=1) as pp,
            tc.tile_pool(name="slab", bufs=3) as slp,
            tc.tile_pool(name="stage", bufs=3) as sp,
        ):
            idx_t = pp.tile([128, KS * PPC // 16], mybir.dt.int16, tag="idx")
            bd_t = pp.tile([128, 2 * KS, 128], mybir.dt.bfloat16, tag="bd")
            ident = pp.tile([128, 128], mybir.dt.bfloat16, tag="ident")
            G = pp.tile([128, KS, 2, NF, PPC // 2], mybir.dt.bfloat16,
                        tag="G")
            # SBUF-resident token table, 2-stripe layout (see below)
            T = pp.tile([128, P // 64, NBC // 2], mybir.dt.bfloat16,
                        tag="T")

            nc.sync.dma_start(idx_t[:], idx_ext[:, :])
            nc.sync.dma_start(bd_t[:], wbd_ext[:, :].rearrange(
                "p (a b) -> p a b", b=128))

            from concourse.masks import make_identity
            make_identity(nc, ident[:])

            # token table, 2-stripe layout: token q is split across TWO
            # partitions so the gather reads two SBUF ports in parallel:
            #   stripe st in {0,1}: T[st*64 + q%64, q//64, e] =
            #       x[bc = st*512 + e, q]   (1KB per stripe, rank = q//64)
            # The f32->bf16 cast rides the input DMA (SWDGE cast); PE
            # transposes (128, 64) blocks to psum partition base st*64.
            with tc.tile_pool(name="pstr", bufs=4, space="PSUM") as ptr:
                for s in range(NSLAB):
                    st64 = (s // 4) * 64       # stripe partition base
                    eoff = (s % 4) * 128       # e-offset within stripe
                    Xs = slp.tile([128, P], mybir.dt.bfloat16, tag="Xs")
                    nc.gpsimd.dma_start(Xs[:],
                                        x_ext[s * 128:(s + 1) * 128, :])
                    for g in range(16):
                        pt = ptr.tile([128, 4, 128], mybir.dt.bfloat16,
                                      tag="pt")
                        for r4 in range(4):
                            b64 = g * 4 + r4   # 64-pixel block = rank
                            nc.tensor.transpose(
                                pt[st64:st64 + 64, r4, :],
                                Xs[:, b64 * 64:(b64 + 1) * 64],
                                ident[:])
                        nc.vector.tensor_copy(
                            out=T[st64:st64 + 64, g * 4:(g + 1) * 4,
                                  eoff:eoff + 128],
                            in_=pt[st64:st64 + 64, :, :])


            # gather: 18 calls of 256 idxs (2 stripes double the per-call
            # descriptor count; the ring caps at ~1K descriptors)
            HPC = PPC // 2
            for k in range(KS):
                for h in range(2):
                    c = 2 * k + h
                    nc.gpsimd.dma_gather(
                        G[:, k, h, :, :],
                        T[:].rearrange("p r e -> p (r e)"),
                        idx_t[:, c * (HPC // 16):(c + 1) * (HPC // 16)],
                        HPC,        # num_idxs
                        HPC,        # num_idxs_reg (all valid)
                        NBC,        # elem_size (bf16 elements = 2KB)
                        transpose=True,
                        sbuf_tokens_per_rank=64,
                        sbuf_free_dim_per_rank=NBC,  # payload B per rank
                        queue_num=c % NQUEUES,
                    )

            # Keep the PE's HAM clock warm through the gather window: the
            # PE idles ~65..85us otherwise and drops to 1.2GHz for the
            # matmul phase.  Dummy transposes read the last table block so
            # they become runnable exactly when the build finishes.
            with tc.tile_pool(name="pswarm", bufs=1, space="PSUM") as pw:
                junk = pw.tile([128, 128], mybir.dt.bfloat16, tag="junk")
                for _ in range(40):
                    nc.tensor.transpose(junk[:], T[:, 63, 384:512],
                                        ident[:])

            # matmuls: batch group f, pair bp -> batches 4f+2bp+{0,1}.
            # f-major so each (f, bp) group's PSUM copy + out DMA stagger
            # into the matmul stream instead of bunching at the end.
            with tc.tile_pool(name="psmm", bufs=6, space="PSUM") as pmm:
                for f in range(NF):
                    for bp in range(2):
                        ps = pmm.tile([128, PPC], mybir.dt.float32,
                                      tag="ps_mm")
                        for k in range(KS):
                            nc.tensor.matmul(
                                ps[:],
                                bd_t[:, bp * KS + k, :],
                                G[:, k, :, f, :],
                                start=(k == 0),
                                stop=(k == KS - 1),
                            )
                        st = sp.tile([128, PPC], mybir.dt.float32,
                                     tag="st")
                        nc.vector.tensor_copy(out=st[:], in_=ps[:])
                        row = (f * 2 + bp) * 128
                        nc.sync.dma_start(out_ext[row:row + 128, :], st[:])

    nc.compile()
    return nc


def _get_nc():
    if "nc" not in _cache:
        _cache["nc"] = _build()
    return _cache["nc"]


def _prep_idx16(idx: np.ndarray) -> list:
    """idx (1,64,64,9) int32 -> per-core (128, KS*PPC//16) int16 lists.

    Core i handles pixels [PPC*i, PPC*(i+1)).  Chunk k holds idx[p, k] for
    those pixels, wrapped: element j at partition j%16, col j//16
    (replicated to the 8 16-partition groups)."""
    lst = idx.reshape(P, KS).astype(np.int16)
    hpc = PPC // 2
    outs = []
    for i in range(NCORES):
        o = np.zeros((128, KS * (PPC // 16)), dtype=np.int16)
        for k in range(KS):
            for h in range(2):
                c = 2 * k + h
                lo = PPC * i + h * hpc
                w = lst[lo:lo + hpc, k].reshape(hpc // 16, 16).T
                o[:, c * (hpc // 16):(c + 1) * (hpc // 16)] = \
                    np.tile(w, (8, 1))
        outs.append(o)
    return outs


def _prep_wbd(weights: np.ndarray) -> np.ndarray:
    """weights (64, 288) f32 -> block-diag lhsT set (128, 2*KS*128) bf16.

    BD[bp, k][32*b' + c, 64*j + m] = W[m, c*KS+k] if b' == 2*bp+j else 0,
    for b' in 0..4 (batch-within-group); reused for every group f."""
    bd = np.zeros((2, KS, 128, 128), dtype=np.float32)
    for k in range(KS):
        wk = weights[:, k::KS]  # (64, 32) = W[m, c*KS+k]
        for bp in range(2):
            for j in range(2):
                bprime = 2 * bp + j
                bd[bp, k, 32 * bprime:32 * bprime + 32, 64 * j:64 * j + 64] = \
                    wk.T
    return bd.reshape(2 * KS, 128, 128).transpose(1, 0, 2).reshape(
        128, 2 * KS * 128).astype(ml_dtypes.bfloat16)


def prep_in_maps(x: np.ndarray, weights: np.ndarray, idx: np.ndarray):
    idx16s = _prep_idx16(np.asarray(idx))
    wbd = _prep_wbd(np.asarray(weights, dtype=np.float32))
    xf = np.ascontiguousarray(
        np.asarray(x, dtype=np.float32).reshape(NBC, P))
    return [{"x": xf, "wbd": wbd, "idx16": idx16s[i]} for i in range(NCORES)]


def assemble_out(results) -> np.ndarray:
    out = np.empty((B, K, P), dtype=np.float32)
    for i in range(NCORES):
        r = np.asarray(results[i]["out"]).astype(np.float32).reshape(
            NF, 2, 2, K, PPC)  # (f, bp, j, m, p)
        for f in range(NF):
            for bp in range(2):
                for j in range(2):
                    out[4 * f + 2 * bp + j, :, PPC * i:PPC * (i + 1)] = \
                        r[f, bp, j]
    return out.reshape(B, K, H, W_IMG)


last_results = None


def kernel(x, weights, idx):
    global last_results
    nc = _get_nc()
    in_maps = prep_in_maps(x, weights, idx)
    trace = bool(int(os.environ.get("KERNEL_TRACE", "0")))
    res = run_bass_kernel_spmd(nc, in_maps, core_ids=list(range(NCORES)),
                               trace=trace)
    last_results = res
    return assemble_out(res.results)
